# revision 9
# baseline (speedup 1.0000x reference)
"""MoE (8 experts, top-2) TRN2 kernel — expert-parallel, dense-masked variant.

Core i holds expert i's weights (bf16); x replicated (fp32 transposed for the
fp32 gating matmul + bf16 transposed for the FFN). Each core computes fp32
gating for all tokens, derives its expert's top-2-masked softmax weight
comb_e[t], runs the FFN on ALL tokens in bf16, scales rows by comb_e and
writes a partial output. Host sums the 8 partials.

Gating columns are permuted per core so "my expert" is always column 0.
"""

import sys
import types

sys.path.insert(0, "/opt/trn_rl_repo")

import numpy as np
import ml_dtypes

try:
    import antenv.axon_hooks  # noqa: F401
except ImportError:
    try:
        import antenv
        import trn_agent_boot.trn_boot as _tb

        _hook = _tb._ntff_profile_via_ctypes("/opt/axon/libaxon_pjrt.so")
        _m = types.ModuleType("antenv.axon_hooks")
        _m.get_axon_ntff_profile_hook = lambda: _hook
        _m.set_axon_ntff_profile_hook = lambda h: None
        sys.modules["antenv.axon_hooks"] = _m
        antenv.axon_hooks = _m
    except Exception:
        pass

import concourse.bacc as bacc
import concourse.mybir as mybir
from concourse import bass, bass_utils
from concourse.tile import TileContext
from concourse.masks import make_identity

E = 8
H = 512
F = 2048
T = 8 * 2048
BF16 = mybir.dt.bfloat16
F32 = mybir.dt.float32

_CACHE = {}
LAST_RESULT = None


def _build():
    nc = bacc.Bacc(debug=False)

    xt = nc.declare_dram_parameter("xt", [128, 4, T], F32, isOutput=False)
    xbt = nc.declare_dram_parameter("xbt", [128, 4, T], BF16, isOutput=False)
    wg = nc.declare_dram_parameter("wg", [128, 4, E], F32, isOutput=False)
    bg = nc.declare_dram_parameter("bg", [E, 1], F32, isOutput=False)
    w1 = nc.declare_dram_parameter("w1", [128, 4, F], BF16, isOutput=False)
    b1t = nc.declare_dram_parameter("b1t", [128, F // 128], F32, isOutput=False)
    w2 = nc.declare_dram_parameter("w2", [128, F // 128, H], BF16, isOutput=False)
    b2r = nc.declare_dram_parameter("b2r", [128, H], F32, isOutput=False)
    ypart = nc.declare_dram_parameter("ypart", [T, H], F32, isOutput=True)

    with TileContext(nc) as tc:
        with (
            tc.tile_pool(name="const", bufs=1) as constp,
            tc.tile_pool(name="work", bufs=3) as work,
            tc.tile_pool(name="gate", bufs=3) as gate,
            tc.tile_pool(name="big", bufs=1) as bigp,
            tc.tile_pool(name="psA", bufs=2, space="PSUM") as psA,
            tc.tile_pool(name="psB", bufs=2, space="PSUM") as psB,
            tc.tile_pool(name="psT", bufs=4, space="PSUM") as psT,
        ):
            ident = constp.tile([128, 128], F32)
            make_identity(nc, ident[:])
            wg_sb = constp.tile([128, 4, E], F32)
            nc.sync.dma_start(out=wg_sb[:], in_=wg[:])
            bg_sb = constp.tile([E, 1], F32)
            nc.sync.dma_start(out=bg_sb[:], in_=bg[:])
            w1_sb = constp.tile([128, 4, F], BF16)
            nc.sync.dma_start(out=w1_sb[:], in_=w1[:])
            b1_sb = constp.tile([128, F // 128], F32)
            nc.sync.dma_start(out=b1_sb[:], in_=b1t[:])
            w2_sb = constp.tile([128, F // 128, H], BF16)
            nc.sync.dma_start(out=w2_sb[:], in_=w2[:])
            b2_sb = constp.tile([128, H], F32)
            nc.sync.dma_start(out=b2_sb[:], in_=b2r[:])

            comb_all = bigp.tile([128, 128], F32)  # [token%128, token//128]

            # ---- gating (fp32) + top-2 routing, per 2048-token group
            for og in range(T // 2048):
                lsbs = []
                for sg in range(4):
                    g = og * 4 + sg
                    xt_sb = gate.tile([128, 4, 512], F32, tag="xt")
                    for c in range(4):
                        nc.sync.dma_start(
                            out=xt_sb[:, c, :], in_=xt[:, c, g * 512 : (g + 1) * 512]
                        )
                    lp = psA.tile([E, 512], F32, tag="mmA")
                    for c in range(4):
                        nc.tensor.matmul(
                            lp[:],
                            wg_sb[:, c, :],
                            xt_sb[:, c, :],
                            start=(c == 0),
                            stop=(c == 3),
                        )
                    l_sb = gate.tile([E, 512], F32, tag="lsb")
                    nc.vector.tensor_scalar_add(l_sb[:], lp[:], bg_sb[:, 0:1])
                    lsbs.append(l_sb)
                lt = gate.tile([128, 16, E], F32, tag="lt")
                for k in range(16):
                    tp = psT.tile([128, E], F32, tag="tp")
                    nc.tensor.transpose(
                        tp[:],
                        lsbs[k // 4][:, (k % 4) * 128 : (k % 4 + 1) * 128],
                        ident[:E, :E],
                    )
                    nc.vector.tensor_copy(out=lt[:, k, :], in_=tp[:])
                m1 = gate.tile([128, 16], F32, tag="m1")
                nc.vector.tensor_reduce(
                    m1[:], lt[:], axis=mybir.AxisListType.X, op=mybir.AluOpType.max
                )
                lsh = gate.tile([128, 16, E], F32, tag="lsh")
                nc.vector.tensor_tensor(
                    out=lsh[:],
                    in0=lt[:],
                    in1=m1[:].to_broadcast([128, 16, E]),
                    op=mybir.AluOpType.subtract,
                )
                ex = gate.tile([128, 16, E], F32, tag="ex")
                nc.scalar.activation(ex[:], lsh[:], mybir.ActivationFunctionType.Exp)
                ssum = gate.tile([128, 16], F32, tag="ssum")
                nc.vector.tensor_reduce(
                    ssum[:], ex[:], axis=mybir.AxisListType.X, op=mybir.AluOpType.add
                )
                rcp = gate.tile([128, 16], F32, tag="rcp")
                nc.vector.reciprocal(rcp[:], ssum[:])
                eq = gate.tile([128, 16, E], F32, tag="eq")
                nc.vector.tensor_scalar(
                    eq[:], lsh[:], 0.0, None, op0=mybir.AluOpType.is_ge
                )
                msk = gate.tile([128, 16, E], F32, tag="msk")
                nc.vector.scalar_tensor_tensor(
                    out=msk[:],
                    in0=eq[:],
                    scalar=-1e30,
                    in1=lsh[:],
                    op0=mybir.AluOpType.mult,
                    op1=mybir.AluOpType.add,
                )
                t2 = gate.tile([128, 16], F32, tag="t2")
                nc.vector.tensor_reduce(
                    t2[:], msk[:], axis=mybir.AxisListType.X, op=mybir.AluOpType.max
                )
                sel = gate.tile([128, 16, E], F32, tag="sel")
                nc.vector.tensor_tensor(
                    out=sel[:],
                    in0=lsh[:],
                    in1=t2[:].to_broadcast([128, 16, E]),
                    op=mybir.AluOpType.is_ge,
                )
                pm = gate.tile([128, 16, E], F32, tag="pm")
                nc.vector.tensor_tensor(
                    out=pm[:], in0=ex[:], in1=sel[:], op=mybir.AluOpType.mult
                )
                cmb = gate.tile([128, 16, E], F32, tag="cmb")
                nc.vector.tensor_tensor(
                    out=cmb[:],
                    in0=pm[:],
                    in1=rcp[:].to_broadcast([128, 16, E]),
                    op=mybir.AluOpType.mult,
                )
                nc.vector.tensor_copy(
                    out=comb_all[:, og * 16 : (og + 1) * 16], in_=cmb[:, :, 0]
                )

            # ---- FFN (bf16) over all T tokens in groups of 512
            for g in range(T // 512):
                xg_sb = work.tile([128, 4, 512], BF16, tag="xg")
                for c in range(4):
                    nc.sync.dma_start(
                        out=xg_sb[:, c, :], in_=xbt[:, c, g * 512 : (g + 1) * 512]
                    )
                hb = work.tile([128, F // 128, 512], BF16, tag="hb")
                for ft in range(F // 128):
                    hp = psA.tile([128, 512], F32, tag="mmA")
                    for hc in range(4):
                        nc.tensor.matmul(
                            hp[:],
                            w1_sb[:, hc, ft * 128 : (ft + 1) * 128],
                            xg_sb[:, hc, :],
                            start=(hc == 0),
                            stop=(hc == 3),
                        )
                    nc.scalar.activation(
                        hb[:, ft, :],
                        hp[:],
                        mybir.ActivationFunctionType.Gelu_apprx_tanh,
                        bias=b1_sb[:, ft : ft + 1],
                        scale=1.0,
                    )
                y_sb = work.tile([128, 4, H], F32, tag="ysb")
                for ht in range(4):
                    yp = psB.tile([128, 512], F32, tag="mmB")
                    for fc in range(F // 128):
                        nc.tensor.matmul(
                            yp[:],
                            w2_sb[:, fc, ht * 128 : (ht + 1) * 128],
                            hb[:, fc, :],
                            start=(fc == 0),
                            stop=(fc == F // 128 - 1),
                        )
                    yt_sb = work.tile([128, 512], F32, tag="ytsb")
                    nc.vector.tensor_copy(out=yt_sb[:], in_=yp[:])
                    for st in range(4):
                        ypt = psT.tile([128, 128], F32, tag="tp")
                        nc.tensor.transpose(
                            ypt[:], yt_sb[:, st * 128 : (st + 1) * 128], ident[:]
                        )
                        nc.vector.tensor_tensor(
                            out=y_sb[:, st, ht * 128 : (ht + 1) * 128],
                            in0=ypt[:],
                            in1=b2_sb[:, ht * 128 : (ht + 1) * 128],
                            op=mybir.AluOpType.add,
                        )
                for st in range(4):
                    nc.vector.tensor_scalar_mul(
                        y_sb[:, st, :], y_sb[:, st, :], comb_all[:, 4 * g + st : 4 * g + st + 1]
                    )
                    nc.sync.dma_start(
                        out=ypart[g * 512 + st * 128 : g * 512 + (st + 1) * 128, :],
                        in_=y_sb[:, st, :],
                    )
    nc.compile()
    return nc


def _prep_inputs(x, Wg, bg, W1, b1, W2, b2):
    xf = np.ascontiguousarray(np.asarray(x, dtype=np.float32).reshape(T, H))
    Wg = np.asarray(Wg, dtype=np.float32)
    bg = np.asarray(bg, dtype=np.float32)
    W1 = np.asarray(W1, dtype=np.float32)
    b1 = np.asarray(b1, dtype=np.float32)
    W2 = np.asarray(W2, dtype=np.float32)
    b2 = np.asarray(b2, dtype=np.float32)

    xtq = np.ascontiguousarray(np.transpose(xf.T.reshape(4, 128, T), (1, 0, 2)))
    xbt = np.ascontiguousarray(xtq.astype(ml_dtypes.bfloat16))

    in_maps = []
    for e in range(E):
        perm = [e] + [j for j in range(E) if j != e]
        wg_p = Wg[:, perm]
        bg_p = bg[perm]
        in_maps.append(
            {
                "xt": xtq,
                "xbt": xbt,
                "wg": np.ascontiguousarray(
                    np.transpose(wg_p.reshape(4, 128, E), (1, 0, 2))
                ),
                "bg": np.ascontiguousarray(bg_p.reshape(E, 1)),
                "w1": np.ascontiguousarray(
                    np.transpose(W1[e].reshape(4, 128, F), (1, 0, 2)).astype(
                        ml_dtypes.bfloat16
                    )
                ),
                "b1t": np.ascontiguousarray(b1[e].reshape(F // 128, 128).T),
                "w2": np.ascontiguousarray(
                    np.transpose(W2[e].reshape(F // 128, 128, H), (1, 0, 2)).astype(
                        ml_dtypes.bfloat16
                    )
                ),
                "b2r": np.ascontiguousarray(
                    np.broadcast_to(b2[e][None, :], (128, H)).copy()
                ),
            }
        )
    return in_maps


def kernel(x, Wg, bg, W1, b1, W2, b2):
    global LAST_RESULT
    if "nc" not in _CACHE:
        _CACHE["nc"] = _build()
    nc = _CACHE["nc"]
    in_maps = _prep_inputs(x, Wg, bg, W1, b1, W2, b2)
    import os

    trace = bool(os.environ.get("BASS_TRACE"))
    res = bass_utils.run_bass_kernel_spmd(
        nc, in_maps, core_ids=list(range(E)), trace=trace
    )
    LAST_RESULT = res
    out = res.results[0]["ypart"].astype(np.float64)
    for e in range(1, E):
        out += res.results[e]["ypart"].astype(np.float64)
    return out.astype(np.float32).reshape(8, 2048, H)
